# revision 20
# baseline (speedup 1.0000x reference)
"""HalfEdgeConv Trainium2 kernel.

out[e] = relu(W @ concat(x[next_idx[e]], has_twin[e] ? x[twin_idx[e]] : 0) + b)

Strategy (data-parallel over half-edges, 8 cores):
  - x is padded with one zero row (index N); dead twins are redirected to it
    on the host, so the device kernel has no masking to do.
  - Per 128-edge tile: two indirect DMA gathers (one row per partition) build
    cat = [128 edges, 128 ch] in SBUF; PE transposes it to channel-major;
    PE matmul with stationary activations produces [128 edges, 64] in PSUM;
    DVE adds the (pre-broadcast) bias, ACT applies ReLU into SBUF; HWDGE
    stores the tile contiguously.
"""
import os
import sys

sys.path.insert(0, "/opt/trn_rl_repo")

import numpy as np
from contextlib import ExitStack

import concourse.bass as bass
import concourse.tile as tile
from concourse import bacc, mybir, bass_utils

N = 1_000_000
C = 64
NCORES = 8
P = 128
TILES = 992                 # 128-edge tiles per core
EPC = P * TILES             # 126976 edges per core
NPAD = NCORES * EPC         # 1015808 padded edges

f32 = mybir.dt.float32
i32 = mybir.dt.int32

_COMPILED = None
LAST_EXEC_NS = None


def _try_install_ntff_shim():
    """NTFF profiling hook (trace runs only); degrade silently if absent."""
    import types, ctypes, contextlib
    if "antenv.axon_hooks" in sys.modules:
        return
    try:
        import antenv
        mod = types.ModuleType("antenv.axon_hooks")
        mod._hook = None
        mod.set_axon_ntff_profile_hook = lambda h: setattr(mod, "_hook", h)
        mod.get_axon_ntff_profile_hook = lambda: mod._hook
        sys.modules["antenv.axon_hooks"] = mod
        antenv.axon_hooks = mod
        lib = ctypes.CDLL("/opt/axon/libaxon_pjrt.so")
        if not hasattr(lib, "axon_start_nrt_profile"):
            return
        lib.axon_start_nrt_profile.argtypes = [ctypes.POINTER(ctypes.c_int64), ctypes.c_size_t]
        lib.axon_start_nrt_profile.restype = ctypes.c_int64
        lib.axon_stop_nrt_profile.argtypes = [ctypes.c_char_p]
        lib.axon_stop_nrt_profile.restype = ctypes.c_int64

        @contextlib.contextmanager
        def _hook(output_dir, device_ids):
            import jax
            jax.devices()
            if device_ids:
                ids = (ctypes.c_int64 * len(device_ids))(*device_ids)
                rc = lib.axon_start_nrt_profile(ids, len(device_ids))
            else:
                rc = lib.axon_start_nrt_profile(None, 0)
            if rc != 0:
                raise RuntimeError(f"axon_start_nrt_profile rc={rc}")
            try:
                yield
            finally:
                lib.axon_stop_nrt_profile(str(output_dir).encode())

        mod.set_axon_ntff_profile_hook(_hook)
    except Exception:
        pass


def _build():
    nc = bacc.Bacc("TRN2", target_bir_lowering=False, debug=False)
    x_d = nc.dram_tensor("x", [N + 1, C], f32, kind="ExternalInput").ap()
    ni_d = nc.dram_tensor("nidx", [P, TILES], i32, kind="ExternalInput").ap()
    ti_d = nc.dram_tensor("tidx", [P, TILES], i32, kind="ExternalInput").ap()
    wt_d = nc.dram_tensor("wt", [2 * C, C], f32, kind="ExternalInput").ap()
    b_d = nc.dram_tensor("bias", [P, C], f32, kind="ExternalInput").ap()
    id_d = nc.dram_tensor("ident", [P, P], f32, kind="ExternalInput").ap()
    out_d = nc.dram_tensor("out", [TILES, P, C], f32, kind="ExternalOutput").ap()

    with tile.TileContext(nc) as tc:
        with ExitStack() as ctx:
            const = ctx.enter_context(tc.tile_pool(name="const", bufs=1))
            catp = ctx.enter_context(tc.tile_pool(name="cat", bufs=8))
            actp = ctx.enter_context(tc.tile_pool(name="act", bufs=4))
            outp = ctx.enter_context(tc.tile_pool(name="outp", bufs=8))
            ptp = ctx.enter_context(tc.tile_pool(name="pt", bufs=3, space="PSUM"))
            pop = ctx.enter_context(tc.tile_pool(name="po", bufs=3, space="PSUM"))

            wt_sb = const.tile([2 * C, C], f32)
            nc.sync.dma_start(wt_sb[:], wt_d[:])
            b_sb = const.tile([P, C], f32)
            nc.sync.dma_start(b_sb[:], b_d[:])
            id_sb = const.tile([P, P], f32)
            nc.sync.dma_start(id_sb[:], id_d[:])
            ni_sb = const.tile([P, TILES], i32)
            nc.sync.dma_start(ni_sb[:], ni_d[:])
            ti_sb = const.tile([P, TILES], i32)
            nc.sync.dma_start(ti_sb[:], ti_d[:])

            for t in range(TILES):
                cat = catp.tile([P, 2, C], f32, tag="cat")
                nc.gpsimd.indirect_dma_start(
                    out=cat[:, 0, :], out_offset=None, in_=x_d[:],
                    in_offset=bass.IndirectOffsetOnAxis(ap=ni_sb[:, t:t + 1], axis=0))
                nc.gpsimd.indirect_dma_start(
                    out=cat[:, 1, :], out_offset=None, in_=x_d[:],
                    in_offset=bass.IndirectOffsetOnAxis(ap=ti_sb[:, t:t + 1], axis=0))

                pt = ptp.tile([P, P], f32, tag="pt")
                nc.tensor.transpose(out=pt[:], in_=cat[:, :, :], identity=id_sb[:])
                actT = actp.tile([P, P], f32, tag="actT")
                nc.vector.tensor_copy(actT[:], pt[:])

                po = pop.tile([P, C], f32, tag="po")
                nc.tensor.matmul(out=po[:], lhsT=actT[:], rhs=wt_sb[:],
                                 start=True, stop=True)
                nc.vector.tensor_add(out=po[:], in0=po[:], in1=b_sb[:])
                ot = outp.tile([P, C], f32, tag="ot")
                nc.scalar.activation(ot[:], po[:], mybir.ActivationFunctionType.Relu)
                nc.sync.dma_start(out_d[t, :, :], ot[:])

    nc.compile()
    return nc


def _get_compiled():
    global _COMPILED
    if _COMPILED is None:
        _COMPILED = _build()
    return _COMPILED


def kernel(x, next_idx, twin_idx, has_twin, W, b):
    global LAST_EXEC_NS
    x = np.asarray(x, dtype=np.float32)
    next_idx = np.asarray(next_idx, dtype=np.int32)
    twin_idx = np.asarray(twin_idx, dtype=np.int32)
    has_twin = np.asarray(has_twin)
    W = np.asarray(W, dtype=np.float32)
    b = np.asarray(b, dtype=np.float32)

    trace = bool(os.environ.get("BASS_TRACE"))
    if trace:
        _try_install_ntff_shim()

    # Host-side input prep: pad x with a zero row; dead twins -> zero row.
    x_pad = np.concatenate([x, np.zeros((1, C), np.float32)], axis=0)
    npad = np.zeros(NPAD, np.int32)
    npad[:N] = next_idx
    tpad = np.full(NPAD, N, np.int32)
    tpad[:N] = np.where(has_twin, twin_idx, N).astype(np.int32)

    wt = np.ascontiguousarray(W.T)                      # [128, 64]
    bias = np.broadcast_to(b, (P, C)).copy()            # [128, 64]
    ident = np.eye(P, dtype=np.float32)

    in_maps = []
    for c in range(NCORES):
        sl = slice(c * EPC, (c + 1) * EPC)
        # idx_sb[p, t] = edge (t*128 + p) of this core's slice
        ni = np.ascontiguousarray(npad[sl].reshape(TILES, P).T)
        ti = np.ascontiguousarray(tpad[sl].reshape(TILES, P).T)
        in_maps.append({"x": x_pad, "nidx": ni, "tidx": ti,
                        "wt": wt, "bias": bias, "ident": ident})

    nc = _get_compiled()
    res = bass_utils.run_bass_kernel_spmd(
        nc, in_maps, core_ids=list(range(NCORES)), trace=trace)
    LAST_EXEC_NS = res.exec_time_ns

    out = np.concatenate(
        [res.results[c]["out"].reshape(EPC, C) for c in range(NCORES)], axis=0)
    return out[:N]

